# revision 1
# baseline (speedup 1.0000x reference)
"""Causal self-attention Trainium2 kernel (8-core SPMD), v2.

Problem: x[2,2048,1024], causal mask, Wqkv[3072,1024], Wo[1024,1024], fp32.
  qkv = x @ Wqkv.T ; per-head causal softmax attention ; out = attn @ Wo.T

Sharding (data + tensor parallel over heads):
  core c -> batch b = c // 4, heads {4g..4g+3} with g = c % 4.
  Each core computes Q,K,V for its 4 heads, runs causal attention, applies
  the matching 256 columns of Wo, and writes a partial [2048,1024] output;
  the host sums the 4 partials per batch (the tensor-parallel reduction).

Design (trace-driven; baseline 189.7us -> ~164us):
  - Attention math: scores computed transposed (scoresT[k,q], head pairs
    packed via PE partition-base tiling) so AV needs no transposes; 2-bank
    PSUM score tiles (two k-blocks) so one ACT exp covers 1024 columns; V
    carries a ones column so AV's 65th partition accumulates the softmax
    denominator for free; normalization via fast reciprocal + gpsimd
    partition broadcast; causal masking by block skipping + one binary
    tril multiply per 128x128 diagonal block. All-bf16 compute, fp32 PSUM.
  - ACT exp is the cadence limiter inside attention (~1.11us per 1024 cols,
    ~0.26us fixed + 0.83ns/col, vs ~0.87us of PE work per score pair). All
    PE work without an ACT dependency (projection chains, output-projection
    units) is emitted as small filler units WOVEN between score pairs so
    the PE keeps running while exps drain: an explicit 16-section schedule
    assigns each (q-chunk, head) section its filler list, with the qc2/qc3
    sections interleaved so the tail-gating exps start earlier.
  - Projections are compact sequential 8-matmul chains into 1-bank PSUM
    tiles (live ~2us each), so PSUM fits: 2x[128,1024] score tiles +
    2x[128,512] chain/outproj tiles + 2x[65,512] AV tiles = 8 banks.
  - DMA in first-needed order as [128,512] pieces over 3 engine queues
    (sync/gpsimd/scalar): wqk, x[qc0], wv, mask, x[qc1..2], wo, x[qc3].
  - Rejected after measurement: fp8 DoubleRow for the qk projection (2.5x
    per-matmul, -7us PE busy, but e2e-neutral because ACT-gating absorbs
    it, while error rises 3.3e-3 -> 1.72e-2); fp8 anywhere else (fails the
    2e-2 budget); bf16 output (PSUM->bf16 DVE cast is 2x the f32 copy
    cost on the tail critical path); DoublePixel/DoubleColumn (no HW
    speedup); wider 2048-col exps (need 4 PSUM banks -> pool starvation).
"""

import numpy as np

S = 2048
D = 1024
DH = 64
B = 2
NCORES = 8
HPC = 4  # heads per core
QKC = 2 * HPC * DH  # 512 q+k projection columns per core
VC = HPC * DH  # 256 v columns per core
P = 128
KO = D // P  # 8 contraction tiles
NQ = S // 512  # 4 q-chunks of 512

_cache = {}


def _build():
    import concourse.bacc as bacc
    import concourse.mybir as mybir
    import concourse.tile as tile

    F32 = mybir.dt.float32
    CDT = mybir.dt.bfloat16
    EXP = mybir.ActivationFunctionType.Exp

    nc = bacc.Bacc()
    xT_d = nc.dram_tensor("xT", [D, S], CDT, kind="ExternalInput")
    wqkT_d = nc.dram_tensor("wqkT", [D, QKC], CDT, kind="ExternalInput")
    wvT_d = nc.dram_tensor("wvT", [D, VC], CDT, kind="ExternalInput")
    woT_d = nc.dram_tensor("woT", [VC, D], CDT, kind="ExternalInput")
    maskT_d = nc.dram_tensor("maskT", [P, P], CDT, kind="ExternalInput")
    out_d = nc.dram_tensor("out", [S, D], F32, kind="ExternalOutput")

    with tile.TileContext(nc) as tc:
        with (
            tc.tile_pool(name="persist", bufs=1) as persist,
            tc.tile_pool(name="sb_small", bufs=4) as sb_small,
            tc.tile_pool(name="sb_exp", bufs=6) as sb_exp,
            tc.tile_pool(name="sb_out", bufs=4) as sb_out,
            tc.tile_pool(name="pp_s2", bufs=2, space="PSUM") as pp_s2,
            tc.tile_pool(name="pp_ch", bufs=2, space="PSUM") as pp_ch,
            tc.tile_pool(name="pp_av", bufs=2, space="PSUM") as pp_av,
        ):
            xT_sb = persist.tile([P, KO, S], CDT, tag="xT")
            wqkT_sb = persist.tile([P, KO, QKC], CDT, tag="wqkT")
            wvT_sb = persist.tile([P, KO, VC], CDT, tag="wvT")
            woT_sb = persist.tile([P, 2, D], CDT, tag="woT")
            maskT_sb = persist.tile([P, P], CDT, tag="maskT")
            qkT_sb = persist.tile([P, 4, S], CDT, tag="qkT")
            v_sb = persist.tile([P, 4 * NQ, HPC, DH + 1], CDT, tag="v")
            attn_sb = persist.tile([P, 2, S], CDT, tag="attn")

            # --- input DMAs: [128,512] pieces in first-needed order, 3 queues ---
            qs = [nc.sync, nc.gpsimd, nc.scalar]
            di = 0

            def dq():
                nonlocal di
                e = qs[di % 3]
                di += 1
                return e

            for ko in range(KO):
                dq().dma_start(wqkT_sb[:, ko, :], wqkT_d[ko * P : (ko + 1) * P, :])
            for ko in range(KO):
                dq().dma_start(
                    xT_sb[:, ko, 0:512], xT_d[ko * P : (ko + 1) * P, 0:512]
                )
            for ko in range(KO):
                dq().dma_start(wvT_sb[:, ko, :], wvT_d[ko * P : (ko + 1) * P, :])
            dq().dma_start(maskT_sb[:], maskT_d[:])
            for qc in (1, 2):
                for ko in range(KO):
                    dq().dma_start(
                        xT_sb[:, ko, qc * 512 : (qc + 1) * 512],
                        xT_d[ko * P : (ko + 1) * P, qc * 512 : (qc + 1) * 512],
                    )
            dq().dma_start(woT_sb[:], woT_d.rearrange("(ct p) e -> p ct e", p=P))
            for ko in range(KO):
                dq().dma_start(
                    xT_sb[:, ko, 3 * 512 : 4 * 512],
                    xT_d[ko * P : (ko + 1) * P, 3 * 512 : 4 * 512],
                )

            ones_f32 = sb_small.tile([P, DH], F32, tag="ones")
            nc.vector.memset(ones_f32[:], 1.0)
            nc.vector.tensor_copy(
                out=v_sb[:, :, :, DH],
                in_=ones_f32[:, 0 : 4 * NQ * HPC].rearrange(
                    "p (a b) -> p a b", a=4 * NQ
                ),
            )

            # ---------- filler units (PE work with no ACT dependency) ----------
            def qk_chain(qc, slot):
                ch = pp_ch.tile([P, 512], F32, tag="ch")
                for ko in range(KO):
                    nc.tensor.matmul(
                        ch[:],
                        wqkT_sb[:, ko, slot * P : (slot + 1) * P],
                        xT_sb[:, ko, qc * 512 : (qc + 1) * 512],
                        start=(ko == 0),
                        stop=(ko == KO - 1),
                        skip_group_check=True,
                    )
                nc.vector.tensor_copy(
                    out=qkT_sb[:, slot, qc * 512 : (qc + 1) * 512], in_=ch[:]
                )

            def v_chain(qc, j):
                sc = 4 * qc + j
                ch = pp_ch.tile([P, 512], F32, tag="ch")
                for ko in range(KO):
                    nc.tensor.matmul(
                        ch[:, 0:VC],
                        xT_sb[:, ko, sc * P : (sc + 1) * P],
                        wvT_sb[:, ko, :],
                        start=(ko == 0),
                        stop=(ko == KO - 1),
                        skip_group_check=True,
                    )
                nc.vector.tensor_copy(
                    out=v_sb[:, sc, :, 0:DH],
                    in_=ch[:, 0:VC].rearrange("p (h d) -> p h d", h=HPC),
                )

            def outproj_unit(sc, en, tail=False):
                ps_o = pp_ch.tile([P, 512], F32, tag="ch")
                for ct in range(2):
                    nc.tensor.matmul(
                        ps_o[:],
                        attn_sb[:, ct, sc * P : (sc + 1) * P],
                        woT_sb[:, ct, en * 512 : (en + 1) * 512],
                        start=(ct == 0),
                        stop=(ct == 1),
                        skip_group_check=True,
                    )
                o_sb = sb_out.tile([P, 512], F32, tag="osb")
                nc.vector.tensor_copy(out=o_sb[:], in_=ps_o[:])
                nc.sync.dma_start(
                    out_d[sc * P : (sc + 1) * P, en * 512 : (en + 1) * 512],
                    o_sb[:],
                )

            # ---------- attention for one (qc, head), weaving fillers ----------
            def attention_head(qc, h, fillers, post=99):
                hp = (h % 2) * DH
                mq = h // 2
                nkb = 4 * qc + 4
                avs = []
                pair_idx = 0
                for kb0 in range(0, nkb, 2):
                    ps2 = pp_s2.tile([P, 1024], F32, tag="s2")
                    exp2 = sb_exp.tile([P, 1024], CDT, tag="exp")
                    offs = []
                    for half in (0, 1):
                        kb = kb0 + half
                        m = kb - 4 * qc
                        off = max(0, m) * P
                        offs.append(off)
                        nc.tensor.matmul(
                            ps2[:, half * 512 + off : half * 512 + 512],
                            qkT_sb[hp : hp + DH, 2 + mq, kb * P : (kb + 1) * P],
                            qkT_sb[
                                hp : hp + DH, mq, qc * 512 + off : (qc + 1) * 512
                            ],
                            start=True,
                            stop=True,
                            skip_group_check=True,
                        )
                    if offs[0] == 0 and offs[1] == 0:
                        nc.scalar.activation(exp2[:], ps2[:], EXP, scale=0.125)
                    else:
                        for half, off in enumerate(offs):
                            lo = half * 512 + off
                            nc.scalar.activation(
                                exp2[:, lo : half * 512 + 512],
                                ps2[:, lo : half * 512 + 512],
                                EXP,
                                scale=0.125,
                            )
                    for half, off in enumerate(offs):
                        kb = kb0 + half
                        if kb - 4 * qc >= 0:
                            lo = half * 512 + off
                            nc.vector.tensor_mul(
                                out=exp2[:, lo : lo + P],
                                in0=exp2[:, lo : lo + P],
                                in1=maskT_sb[:],
                            )
                        avs.append((exp2, half * 512 + off, off, kb))
                    pair_idx += 1
                    if pair_idx % 2 == 0 and fillers:
                        fillers.popleft()()
                ps_av = pp_av.tile([DH + 1, 512], F32, tag="av")
                for j, (exp2, lo, off, kb) in enumerate(avs):
                    nc.tensor.matmul(
                        ps_av[:, off:512],
                        v_sb[:, kb, h, :],
                        exp2[:, lo : (lo - off) + 512],
                        start=(j == 0),
                        stop=(j == len(avs) - 1),
                        skip_group_check=True,
                    )
                sums_sb = sb_small.tile([1, 512], F32, tag="sums")
                nc.vector.tensor_copy(out=sums_sb[:], in_=ps_av[DH : DH + 1, :])
                recip_f = sb_small.tile([1, 512], F32, tag="recipf")
                nc.vector.reciprocal_approx_fast(out=recip_f[:], in_=sums_sb[:])
                bc_sb = sb_small.tile([DH, 512], F32, tag="bc")
                nc.gpsimd.partition_broadcast(bc_sb[:], recip_f[:])
                nc.vector.tensor_mul(
                    out=attn_sb[hp : hp + DH, mq, qc * 512 : (qc + 1) * 512],
                    in0=ps_av[0:DH, :],
                    in1=bc_sb[:],
                )
                for _ in range(post):
                    if fillers:
                        fillers.popleft()()

            # ---------- schedule ----------
            from collections import deque

            # bootstrap: projections for qc0
            for slot in range(4):
                qk_chain(0, slot)
            for j in range(4):
                v_chain(0, j)

            def qk_u(qc, s):
                return lambda: qk_chain(qc, s)

            def v_u(qc, j):
                return lambda: v_chain(qc, j)

            def o_u(sc, en):
                return lambda: outproj_unit(sc, en)

            def outs(*scs):
                return [o_u(sc, en) for sc in scs for en in range(2)]

            # (qc, h, fillers): heads of qc2/qc3 interleaved so the last
            # head's exps (which gate the tail) start earlier.
            sched = [
                (0, 0, [qk_u(1, 0), qk_u(1, 1)]),
                (0, 1, [qk_u(1, 2), qk_u(1, 3)]),
                (0, 2, [v_u(1, 0), v_u(1, 1)]),
                (0, 3, [v_u(1, 2), v_u(1, 3)]),
                (1, 0, [qk_u(2, s) for s in range(4)]),
                (1, 1, [v_u(2, j) for j in range(4)]),
                (1, 2, outs(0, 1)),
                (1, 3, outs(2, 3)),
                (2, 0, [qk_u(3, s) for s in range(4)]),
                (2, 1, [v_u(3, j) for j in range(4)]),
                (3, 0, outs(4, 5)),
                (2, 2, outs(6, 7)),
                (3, 1, []),
                (2, 3, []),
                (3, 2, outs(8, 9)),
                (3, 3, []),
            ]
            for qc, h, fl in sched:
                attention_head(qc, h, deque(fl))
            # sc10-11 units emitted here run on the PE while the DVE
            # normalize of the final head drains (fills the tail gap)
            for f in outs(10, 11):
                f()
            # tail: outproj for q-chunks 12..15
            for sc in range(12, 16):
                for en in range(2):
                    outproj_unit(sc, en, tail=True)

    nc.compile()
    return nc


def _get_nc():
    if "nc" not in _cache:
        _cache["nc"] = _build()
    return _cache["nc"]


def _shard(x, mask, Wqkv, Wo):
    import ml_dtypes

    cdt = ml_dtypes.bfloat16
    in_maps = []
    maskT = np.ascontiguousarray((mask[0, 0, :P, :P].T >= 0).astype(cdt))
    for c in range(NCORES):
        b = c // 4
        g = c % 4
        heads = [4 * g + i for i in range(HPC)]
        q_rows = np.concatenate([np.arange(h * DH, (h + 1) * DH) for h in heads])
        k_rows = D + q_rows
        v_rows = 2 * D + q_rows
        qk_rows = np.concatenate([q_rows, k_rows])
        in_maps.append(
            {
                "xT": np.ascontiguousarray(x[b].T.astype(cdt)),
                "wqkT": np.ascontiguousarray(Wqkv[qk_rows, :].T.astype(cdt)),
                "wvT": np.ascontiguousarray(Wqkv[v_rows, :].T.astype(cdt)),
                "woT": np.ascontiguousarray(Wo[:, q_rows].T.astype(cdt)),
                "maskT": maskT,
            }
        )
    return in_maps


def kernel(x, mask, Wqkv, Wo, _trace=False):
    from concourse.bass_utils import run_bass_kernel_spmd

    x = np.asarray(x, dtype=np.float32)
    mask = np.asarray(mask, dtype=np.float32)
    Wqkv = np.asarray(Wqkv, dtype=np.float32)
    Wo = np.asarray(Wo, dtype=np.float32)

    nc = _get_nc()
    in_maps = _shard(x, mask, Wqkv, Wo)
    res = run_bass_kernel_spmd(nc, in_maps, core_ids=list(range(NCORES)), trace=_trace)
    _cache["last_result"] = res

    out = np.zeros((B, S, D), dtype=np.float32)
    for c in range(NCORES):
        out[c // 4] += np.asarray(res.results[c]["out"], dtype=np.float32)
    return out



# revision 13
# speedup vs baseline: 1.1544x; 1.1544x over previous
"""Causal self-attention Trainium2 kernel (8-core SPMD), v2.

Problem: x[2,2048,1024], causal mask, Wqkv[3072,1024], Wo[1024,1024], fp32.
  qkv = x @ Wqkv.T ; per-head causal softmax attention ; out = attn @ Wo.T

Sharding (data + tensor parallel over heads):
  core c -> batch b = c // 4, heads {4g..4g+3} with g = c % 4.
  Each core computes Q,K,V for its 4 heads, runs causal attention, applies
  the matching 256 columns of Wo, and writes a partial [2048,1024] output;
  the host sums the 4 partials per batch (the tensor-parallel reduction).

Design (trace-driven; baseline 189.7us -> ~164us):
  - Attention math: scores computed transposed (scoresT[k,q], head pairs
    packed via PE partition-base tiling) so AV needs no transposes; 2-bank
    PSUM score tiles (two k-blocks) so one ACT exp covers 1024 columns; V
    carries a ones column so AV's 65th partition accumulates the softmax
    denominator for free; normalization via fast reciprocal + gpsimd
    partition broadcast; causal masking by block skipping + one binary
    tril multiply per 128x128 diagonal block. All-bf16 compute, fp32 PSUM.
  - ACT exp is the cadence limiter inside attention (~1.11us per 1024 cols,
    ~0.26us fixed + 0.83ns/col, vs ~0.87us of PE work per score pair). All
    PE work without an ACT dependency (projection chains, output-projection
    units) is emitted as small filler units WOVEN between score pairs so
    the PE keeps running while exps drain: an explicit 16-section schedule
    assigns each (q-chunk, head) section its filler list, with the qc2/qc3
    sections interleaved so the tail-gating exps start earlier.
  - Projections are compact sequential 8-matmul chains into 1-bank PSUM
    tiles (live ~2us each), so PSUM fits: 2x[128,1024] score tiles +
    2x[128,512] chain/outproj tiles + 2x[65,512] AV tiles = 8 banks.
  - DMA in first-needed order as [128,512] pieces over 3 engine queues
    (sync/gpsimd/scalar): wqk, x[qc0], wv, mask, x[qc1..2], wo, x[qc3].
  - Rejected after measurement: fp8 DoubleRow for the qk projection (2.5x
    per-matmul, -7us PE busy, but e2e-neutral because ACT-gating absorbs
    it, while error rises 3.3e-3 -> 1.72e-2); fp8 anywhere else (fails the
    2e-2 budget); bf16 output (PSUM->bf16 DVE cast is 2x the f32 copy
    cost on the tail critical path); DoublePixel/DoubleColumn (no HW
    speedup); wider 2048-col exps (need 4 PSUM banks -> pool starvation).
"""

import numpy as np

S = 2048
D = 1024
DH = 64
B = 2
NCORES = 8
HPC = 4  # heads per core
QKC = 2 * HPC * DH  # 512 q+k projection columns per core
VC = HPC * DH  # 256 v columns per core
P = 128
KO = D // P  # 8 contraction tiles
NQ = S // 512  # 4 q-chunks of 512

_cache = {}


def _build():
    import concourse.bacc as bacc
    import concourse.mybir as mybir
    import concourse.tile as tile

    F32 = mybir.dt.float32
    CDT = mybir.dt.bfloat16
    EXP = mybir.ActivationFunctionType.Exp

    nc = bacc.Bacc()
    xT_d = nc.dram_tensor("xT", [D, S], CDT, kind="ExternalInput")
    wqkT_d = nc.dram_tensor("wqkT", [D, QKC], CDT, kind="ExternalInput")
    wvT_d = nc.dram_tensor("wvT", [D, VC], CDT, kind="ExternalInput")
    woT_d = nc.dram_tensor("woT", [VC, D], CDT, kind="ExternalInput")
    maskT_d = nc.dram_tensor("maskT", [P, P], CDT, kind="ExternalInput")
    out_d = nc.dram_tensor("out", [S, D], CDT, kind="ExternalOutput")

    with tile.TileContext(nc) as tc:
        with (
            tc.tile_pool(name="persist", bufs=1) as persist,
            tc.tile_pool(name="sb_small", bufs=4) as sb_small,
            tc.tile_pool(name="sb_exp", bufs=6) as sb_exp,
            tc.tile_pool(name="sb_out", bufs=4) as sb_out,
            tc.tile_pool(name="pp_s2", bufs=2, space="PSUM") as pp_s2,
            tc.tile_pool(name="pp_ch", bufs=2, space="PSUM") as pp_ch,
            tc.tile_pool(name="pp_av", bufs=2, space="PSUM") as pp_av,
        ):
            xT_sb = persist.tile([P, KO, S], CDT, tag="xT")
            wqkT_sb = persist.tile([P, KO, QKC], CDT, tag="wqkT")
            wvT_sb = persist.tile([P, KO, VC], CDT, tag="wvT")
            woT_sb = persist.tile([P, 2, D], CDT, tag="woT")
            maskT_sb = persist.tile([P, P], CDT, tag="maskT")
            qkT_sb = persist.tile([P, 4, S], CDT, tag="qkT")
            v_sb = persist.tile([P, 4 * NQ, HPC, DH + 1], CDT, tag="v")
            attn_sb = persist.tile([P, 2, S], CDT, tag="attn")

            # --- input DMAs: [128,512] pieces in first-needed order, 3 queues,
            # wqk/x0 interleaved per-ko so the first chain starts on piece 0 ---
            qs = [nc.sync, nc.gpsimd, nc.scalar]
            di = 0

            def dq():
                nonlocal di
                e = qs[di % 3]
                di += 1
                return e

            for ko in range(KO):
                dq().dma_start(wqkT_sb[:, ko, :], wqkT_d[ko * P : (ko + 1) * P, :])
                dq().dma_start(
                    xT_sb[:, ko, 0:512], xT_d[ko * P : (ko + 1) * P, 0:512]
                )
            dq().dma_start(maskT_sb[:], maskT_d[:])
            for ko in range(KO):
                dq().dma_start(wvT_sb[:, ko, :], wvT_d[ko * P : (ko + 1) * P, :])
            for qc in (1, 2):
                for ko in range(KO):
                    dq().dma_start(
                        xT_sb[:, ko, qc * 512 : (qc + 1) * 512],
                        xT_d[ko * P : (ko + 1) * P, qc * 512 : (qc + 1) * 512],
                    )
            for ct in range(2):
                dq().dma_start(woT_sb[:, ct, :], woT_d[ct * P : (ct + 1) * P, :])
            for ko in range(KO):
                dq().dma_start(
                    xT_sb[:, ko, 3 * 512 : 4 * 512],
                    xT_d[ko * P : (ko + 1) * P, 3 * 512 : 4 * 512],
                )

            ones_f32 = sb_small.tile([P, DH], F32, tag="ones")
            nc.vector.memset(ones_f32[:], 1.0)
            nc.vector.tensor_copy(
                out=v_sb[:, :, :, DH],
                in_=ones_f32[:, 0 : 4 * NQ * HPC].rearrange(
                    "p (a b) -> p a b", a=4 * NQ
                ),
            )

            # ---------- filler units (PE work with no ACT dependency) ----------
            def qk_chain(qc, slot):
                ch = pp_ch.tile([P, 512], F32, tag="ch")
                for ko in range(KO):
                    nc.tensor.matmul(
                        ch[:],
                        wqkT_sb[:, ko, slot * P : (slot + 1) * P],
                        xT_sb[:, ko, qc * 512 : (qc + 1) * 512],
                        start=(ko == 0),
                        stop=(ko == KO - 1),
                        skip_group_check=True,
                    )
                nc.vector.tensor_copy(
                    out=qkT_sb[:, slot, qc * 512 : (qc + 1) * 512], in_=ch[:]
                )

            def v_chain(qc, j):
                sc = 4 * qc + j
                ch = pp_ch.tile([P, 512], F32, tag="ch")
                for ko in range(KO):
                    nc.tensor.matmul(
                        ch[:, 0:VC],
                        xT_sb[:, ko, sc * P : (sc + 1) * P],
                        wvT_sb[:, ko, :],
                        start=(ko == 0),
                        stop=(ko == KO - 1),
                        skip_group_check=True,
                    )
                nc.vector.tensor_copy(
                    out=v_sb[:, sc, :, 0:DH],
                    in_=ch[:, 0:VC].rearrange("p (h d) -> p h d", h=HPC),
                )

            odma = [0]

            def outproj_unit(sc, en, cp_eng=None, dma_eng=None):
                ps_o = pp_ch.tile([P, 512], F32, tag="ch")
                for ct in range(2):
                    nc.tensor.matmul(
                        ps_o[:],
                        attn_sb[:, ct, sc * P : (sc + 1) * P],
                        woT_sb[:, ct, en * 512 : (en + 1) * 512],
                        start=(ct == 0),
                        stop=(ct == 1),
                        skip_group_check=True,
                    )
                o_sb = sb_out.tile([P, 512], CDT, tag="osb")
                if cp_eng is nc.scalar:
                    nc.scalar.copy(o_sb[:], ps_o[:])
                else:
                    (cp_eng or nc.vector).tensor_copy(out=o_sb[:], in_=ps_o[:])
                if dma_eng is None:
                    dma_eng = [nc.sync, nc.gpsimd][odma[0] % 2]
                    odma[0] += 1
                dma_eng.dma_start(
                    out_d[sc * P : (sc + 1) * P, en * 512 : (en + 1) * 512],
                    o_sb[:],
                )

            # ---------- attention for one (qc, head), weaving fillers ----------
            def attention_head(qc, h, fillers, post=99, drain_before_av=False):
                hp = (h % 2) * DH
                mq = h // 2
                nkb = 4 * qc + 4
                avs = []
                pair_idx = 0
                for kb0 in range(0, nkb, 2):
                    ps2 = pp_s2.tile([P, 1024], F32, tag="s2")
                    exp2 = sb_exp.tile([P, 1024], CDT, tag="exp")
                    offs = []
                    for half in (0, 1):
                        kb = kb0 + half
                        m = kb - 4 * qc
                        off = max(0, m) * P
                        offs.append(off)
                        nc.tensor.matmul(
                            ps2[:, half * 512 + off : half * 512 + 512],
                            qkT_sb[hp : hp + DH, 2 + mq, kb * P : (kb + 1) * P],
                            qkT_sb[
                                hp : hp + DH, mq, qc * 512 + off : (qc + 1) * 512
                            ],
                            start=True,
                            stop=True,
                            skip_group_check=True,
                        )
                    if offs[0] == 0 and offs[1] == 0:
                        nc.scalar.activation(exp2[:], ps2[:], EXP, scale=0.125)
                    else:
                        for half, off in enumerate(offs):
                            lo = half * 512 + off
                            nc.scalar.activation(
                                exp2[:, lo : half * 512 + 512],
                                ps2[:, lo : half * 512 + 512],
                                EXP,
                                scale=0.125,
                            )
                    for half, off in enumerate(offs):
                        kb = kb0 + half
                        if kb - 4 * qc >= 0:
                            lo = half * 512 + off
                            nc.vector.tensor_mul(
                                out=exp2[:, lo : lo + P],
                                in0=exp2[:, lo : lo + P],
                                in1=maskT_sb[:],
                            )
                        avs.append((exp2, half * 512 + off, off, kb))
                    pair_idx += 1
                    if pair_idx % 2 == 0 and fillers:
                        fillers.popleft()()
                if drain_before_av:
                    while fillers:
                        fillers.popleft()()
                ps_av = pp_av.tile([DH + 1, 512], F32, tag="av")
                for j, (exp2, lo, off, kb) in enumerate(avs):
                    nc.tensor.matmul(
                        ps_av[:, off:512],
                        v_sb[:, kb, h, :],
                        exp2[:, lo : (lo - off) + 512],
                        start=(j == 0),
                        stop=(j == len(avs) - 1),
                        skip_group_check=True,
                    )
                sums_sb = sb_small.tile([1, 512], F32, tag="sums")
                nc.vector.tensor_copy(out=sums_sb[:], in_=ps_av[DH : DH + 1, :])
                recip_f = sb_small.tile([1, 512], F32, tag="recipf")
                nc.vector.reciprocal_approx_fast(out=recip_f[:], in_=sums_sb[:])
                bc_sb = sb_small.tile([DH, 512], F32, tag="bc")
                nc.gpsimd.partition_broadcast(bc_sb[:], recip_f[:])
                nc.vector.tensor_mul(
                    out=attn_sb[hp : hp + DH, mq, qc * 512 : (qc + 1) * 512],
                    in0=ps_av[0:DH, :],
                    in1=bc_sb[:],
                )
                for _ in range(post):
                    if fillers:
                        fillers.popleft()()

            # ---------- schedule ----------
            from collections import deque

            # bootstrap: only head0's q (slot0) + k (slot2); the rest of the
            # qc0 projections weave into (0,0)/(0,1) so ACT starts ~12us
            # earlier and stays ahead of the PE for the rest of the kernel.
            qk_chain(0, 0)
            qk_chain(0, 2)

            def qk_u(qc, s):
                return lambda: qk_chain(qc, s)

            def v_u(qc, j):
                return lambda: v_chain(qc, j)

            def o_u(sc, en):
                return lambda: outproj_unit(sc, en)

            def outs(*scs):
                return [o_u(sc, en) for sc in scs for en in range(2)]

            # (qc, h, fillers): heads of qc2/qc3 interleaved so the last
            # head's exps (which gate the tail) start earlier.
            sched = [
                (0, 0, [qk_u(0, 1), qk_u(0, 3)] + [v_u(0, j) for j in range(4)]),
                (0, 1, [qk_u(1, 0), qk_u(1, 2)]),
                (0, 2, [qk_u(1, 1), qk_u(1, 3), v_u(1, 0), v_u(1, 1)]),
                (0, 3, [v_u(1, 2), v_u(1, 3)]),
                (1, 0, [qk_u(2, s) for s in range(4)]),
                (1, 1, [v_u(2, j) for j in range(4)]),
                (1, 2, outs(0, 1)),
                (1, 3, outs(2, 3)),
                (2, 0, [qk_u(3, s) for s in range(4)]),
                (2, 1, [v_u(3, j) for j in range(4)]),
                (3, 0, outs(4, 5)),
                (2, 2, outs(6, 7)),
                (3, 1, []),
                (2, 3, []),
                (3, 2, outs(8)),
                (3, 3, outs(9, 10, 11)),
            ]
            for qc, h, fl in sched:
                attention_head(qc, h, deque(fl), drain_before_av=(qc == 0 and h == 0))
            # tail: outproj for q-chunk 3 regions; copies spread across
            # scalar/vector/gpsimd (all near-idle here), DMA over 2 queues
            cps = [nc.vector, nc.scalar]
            dqs = [nc.sync, nc.gpsimd, nc.scalar]
            ti = 0
            for sc in range(12, 16):
                for en in range(2):
                    outproj_unit(sc, en, cp_eng=cps[ti % 2], dma_eng=dqs[ti % 3])
                    ti += 1

    nc.compile()
    return nc


def _get_nc():
    if "nc" not in _cache:
        _cache["nc"] = _build()
    return _cache["nc"]


def _shard(x, mask, Wqkv, Wo):
    import ml_dtypes

    cdt = ml_dtypes.bfloat16
    in_maps = []
    maskT = np.ascontiguousarray((mask[0, 0, :P, :P].T >= 0).astype(cdt))
    for c in range(NCORES):
        b = c // 4
        g = c % 4
        heads = [4 * g + i for i in range(HPC)]
        q_rows = np.concatenate([np.arange(h * DH, (h + 1) * DH) for h in heads])
        k_rows = D + q_rows
        v_rows = 2 * D + q_rows
        qk_rows = np.concatenate([q_rows, k_rows])
        in_maps.append(
            {
                "xT": np.ascontiguousarray(x[b].T.astype(cdt)),
                "wqkT": np.ascontiguousarray(Wqkv[qk_rows, :].T.astype(cdt)),
                "wvT": np.ascontiguousarray(Wqkv[v_rows, :].T.astype(cdt)),
                "woT": np.ascontiguousarray(Wo[:, q_rows].T.astype(cdt)),
                "maskT": maskT,
            }
        )
    return in_maps


def kernel(x, mask, Wqkv, Wo, _trace=False):
    from concourse.bass_utils import run_bass_kernel_spmd

    x = np.asarray(x, dtype=np.float32)
    mask = np.asarray(mask, dtype=np.float32)
    Wqkv = np.asarray(Wqkv, dtype=np.float32)
    Wo = np.asarray(Wo, dtype=np.float32)

    nc = _get_nc()
    in_maps = _shard(x, mask, Wqkv, Wo)
    res = run_bass_kernel_spmd(nc, in_maps, core_ids=list(range(NCORES)), trace=_trace)
    _cache["last_result"] = res

    out = np.zeros((B, S, D), dtype=np.float32)
    for c in range(NCORES):
        out[c // 4] += np.asarray(res.results[c]["out"]).astype(np.float32)
    return out

